# revision 40
# baseline (speedup 1.0000x reference)
"""BetaGNN message-passing kernel for 8 Trainium2 NeuronCores.

Strategy (1D node partitioning):
  - nodes are sharded across 8 cores (12500 rows/core, padded to 12544 = 98*128)
  - H = relu(X @ W_in + b_in) computed per-shard in feature-major layout,
    transposed on PE into a node-major fp16 gather table, AllGathered to all cores
  - SpMM (y[r] = sum vals[e]*H[col[e]] over edges with row[e]==r): edges are
    chunked 128-at-a-time; source rows are fetched with the GPSIMD dma_gather
    custom op (int16 indices over 4 quarter-tables, 4 SWDGE queues, the kernel's
    bottleneck at ~2.2ns/descriptor); the scaled one-hot selection matrices
    S_T[e,d] = vals[e]*(rowm[e]==d) are HOST-precomputed fp16 and streamed in via
    HWDGE DMA (GPSIMD SWDGE descriptor generation serializes against every DVE op
    on this silicon, so the SpMM inner loop uses no DVE at all); TensorE computes
    G.T @ S_T accumulating AH feature-major in PSUM (one bank per 4 dest tiles)
  - AH is AllGathered (fp16 table) for the second hop; the same S_T stream is
    re-read for hop 2; A2H stays feature-major
  - H2 = relu(W1.T@AH + W2.T@A2H), out = W_out.T@H2 all feature-major in fp16;
    b_out added on host

The edge chunk structure (chunks per (tile, quartile)) is data-dependent and baked
into the compiled program; all 8 cores share one program, so per-(tile,quartile)
chunk counts are the max over cores and each core pads its edge stream with
(idx=0, val=0) slots.
"""

import os
import sys

import numpy as np

if "/opt/trn_rl_repo" not in sys.path:
    sys.path.insert(0, "/opt/trn_rl_repo")

NCORES = 8
P = 128
FJ = 512  # dense-phase column chunk
BANK_TILES = 4  # dest tiles per PSUM bank


QB = 3136  # quarter-block rows per core; table quartile = pos_of // QB


def _balance(row, col, n_nodes, shard, T):
    """Permute nodes across cores/tiles so per-(core,tile,quartile) edge
    counts stay under a shared cap -> chunk counts near the ceil-optimal.

    Table layout: position(k, r) = (r//QB)*8*QB + k*QB + (r%QB), so a node's
    source-quartile is r//QB (its own tile-group). That is circular with the
    balancing itself, so iterate: assign -> recompute quartiles -> reassign.

    Returns (core_of, pos_of): node -> owning core, local slot in [0, T*128).
    """
    deg_in = np.bincount(row, minlength=n_nodes)

    # nodes -> cores: LPT on dest degree with exact shard capacity
    order = np.argsort(-deg_in, kind="stable")
    core_of = np.empty(n_nodes, np.int32)
    core_load = np.zeros(NCORES, np.int64)
    core_cnt = np.zeros(NCORES, np.int64)
    for n in order:
        k = np.argmin(np.where(core_cnt < shard, core_load, 1 << 60))
        core_of[n] = k
        core_load[k] += deg_in[n]
        core_cnt[k] += 1

    # bins: (tile, quart, cap); tiles straddling a QB boundary split in two
    bins = []
    for t in range(T):
        r0, r1 = t * P, (t + 1) * P
        q0, q1 = r0 // QB, (r1 - 1) // QB
        if q0 == q1:
            bins.append((t, q0, P))
        else:
            mid = (q0 + 1) * QB
            bins.append((t, q0, mid - r0))
            bins.append((t, q1, r1 - mid))
    NB = len(bins)
    bin_tile = np.array([b[0] for b in bins])
    bin_quart = np.array([b[1] for b in bins])
    bin_cap = np.array([b[2] for b in bins])
    bin_off = np.empty(NB, np.int64)
    off = 0
    prev_t = -1
    for b in range(NB):
        if bin_tile[b] != prev_t:
            off = 0
            prev_t = bin_tile[b]
        bin_off[b] = off
        off += bin_cap[b]

    quart_of = np.empty(n_nodes, np.int32)
    for k in range(NCORES):
        nodes = np.where(core_of == k)[0]
        quart_of[nodes] = np.arange(len(nodes)) % 4

    best = None
    for _itr in range(3):
        qd_full = np.zeros((n_nodes, 4), np.int64)
        np.add.at(qd_full, (row, quart_of[col]), 1)

        tot_kq = np.zeros((NCORES, 4), np.int64)
        for k in range(NCORES):
            m = core_of[row] == k
            tot_kq[k] = np.bincount(quart_of[col[m]], minlength=4)
        over = np.maximum(0, tot_kq - T * 512)
        n_spill = -(-over.max(axis=0) // P)
        caps = np.full((T, 4), 512, np.int64)
        for q in range(4):
            caps[: n_spill[q], q] = 640

        pos_of = np.empty(n_nodes, np.int32)
        cell_counts = np.zeros((NCORES, T, 4), np.int64)
        new_quart = np.empty(n_nodes, np.int32)
        for k in range(NCORES):
            nodes = np.where(core_of == k)[0]
            d = qd_full[nodes]
            target = d.sum(axis=0) / T
            o = np.argsort(-d.sum(axis=1), kind="stable")
            nodes, d = nodes[o], d[o]
            nk = len(nodes)
            tile_sum = np.zeros((T, 4), np.int64)
            bin_cnt = np.zeros(NB, np.int64)
            assign = np.empty(nk, np.int32)
            for i in range(nk):
                proj = tile_sum[bin_tile] + d[i]
                score = (proj - caps[bin_tile]).max(axis=1).astype(np.float64)
                score += 0.001 * (proj - target).max(axis=1)
                score = np.where(bin_cnt < bin_cap, score, 1e18)
                b = int(np.argmin(score))
                assign[i] = b
                tile_sum[bin_tile[b]] += d[i]
                bin_cnt[b] += 1
            members = [list(np.where(assign == b)[0]) for b in range(NB)]
            tile_bins = [
                [b for b in range(NB) if bin_tile[b] == t] for t in range(T)
            ]
            for _ in range(30000):
                viol = tile_sum - caps
                vq = np.unravel_index(np.argmax(viol), viol.shape)
                if viol[vq] <= 0:
                    break
                t, q = int(vq[0]), int(vq[1])
                done = False
                rows_t = sorted(
                    [(i, b) for b in tile_bins[t] for i in members[b]],
                    key=lambda x: -d[x[0]][q],
                )
                for a, ba in rows_t[:20]:
                    if d[a][q] == 0:
                        break
                    slack = caps[:, q] - tile_sum[:, q]
                    for u in np.argsort(-slack):
                        if slack[u] <= 0:
                            break
                        if u == t:
                            continue
                        for bb in tile_bins[u]:
                            hit = False
                            for bi in members[bb]:
                                if d[bi][q] >= d[a][q]:
                                    continue
                                nt = tile_sum[t] - d[a] + d[bi]
                                nu = tile_sum[u] - d[bi] + d[a]
                                if (
                                    (nt <= caps[t]).all()
                                    or (nt - caps[t]).max() < viol[vq]
                                ) and (nu <= caps[u]).all():
                                    tile_sum[t] = nt
                                    tile_sum[u] = nu
                                    members[ba].remove(a)
                                    members[bb].remove(bi)
                                    members[ba].append(bi)
                                    members[bb].append(a)
                                    hit = True
                                    done = True
                                    break
                            if hit:
                                break
                        if done:
                            break
                    if done:
                        break
                if not done:
                    break
            for b in range(NB):
                base = bin_tile[b] * P + bin_off[b]
                for j, i in enumerate(members[b]):
                    pos_of[nodes[i]] = base + j
                    new_quart[nodes[i]] = bin_quart[b]
            cell_counts[k] = tile_sum
        quart_of = new_quart.copy()
        C = np.maximum(1, -(-cell_counts.max(axis=0) // P))
        S = int(C.sum() * P)
        if best is None or S < best[0]:
            best = (S, pos_of.copy())
    return core_of, best[1]


def _structure(row, col, vals, n_nodes):
    """Shared (cross-core) chunk structure + per-core padded edge arrays."""
    shard = n_nodes // NCORES
    shard_pad = -(-shard // P) * P
    npad = shard_pad * NCORES
    qrows = npad // 4
    assert qrows <= 32768, qrows
    T = shard_pad // P
    banks = [list(range(i, min(i + BANK_TILES, T))) for i in range(0, T, BANK_TILES)]

    core_of, pos_of = _balance(row, col, n_nodes, shard, T)

    core = core_of[row]
    r_loc = pos_of[row]
    q = pos_of[col] // QB
    idxq = (core_of[col] * QB + pos_of[col] % QB).astype(np.int16)
    t = r_loc // P
    rowm = (r_loc % P).astype(np.int32)

    ncells = T * 4
    cell = t * 4 + q
    cnt = np.zeros((NCORES, ncells), np.int64)
    for k in range(NCORES):
        m = core == k
        cnt[k] = np.bincount(cell[m], minlength=ncells)
    C = -(-cnt.max(axis=0) // P)  # chunks per cell (ceil of max count / 128)
    # every tile must own >= 1 chunk so its PSUM window gets written
    for tt in range(T):
        if C[tt * 4 : tt * 4 + 4].sum() == 0:
            C[tt * 4] = 1

    # stream order: for bank, for quartile, for tile-in-bank
    order_cells = [tt * 4 + qq for b in banks for qq in range(4) for tt in b]
    ord_of_cell = np.empty(ncells, np.int64)
    ord_of_cell[np.array(order_cells)] = np.arange(ncells)
    cell_chunks = C[np.array(order_cells)]
    cell_slot0 = np.r_[0, np.cumsum(cell_chunks * P)[:-1]]
    S = int((cell_chunks * P).sum())
    nchunk = S // P

    # gather calls: one per (bank, quartile) with nonzero size
    calls = []  # (quartile, slot_off, size)
    pos = 0
    for b in banks:
        for qq in range(4):
            sz = int(sum(C[tt * 4 + qq] for tt in b)) * P
            if sz:
                calls.append((qq, pos, sz))
                pos += sz
    assert pos == S

    per_core = []
    for k in range(NCORES):
        m = core == k
        ek = ord_of_cell[cell[m]]
        # order slots by (cell stream order, then ascending source idx) --
        # ascending sources improve HBM row locality during the gather
        perm = np.lexsort((idxq[m], ek))
        sorted_ord = ek[perm]
        counts_in_order = cnt[k][np.array(order_cells)]
        run_start = np.r_[0, np.cumsum(counts_in_order)[:-1]]
        rank = np.arange(len(sorted_ord)) - run_start[sorted_ord]
        slot = cell_slot0[sorted_ord] + rank
        idx_slots = np.zeros(S, np.int16)
        idx_slots[slot] = idxq[m][perm]
        # int16 index array: per call, wrap [sz] -> [16, sz/16]; replicate to 128
        idx16 = np.zeros((16, S // 16), np.int16)
        for qq, off, sz in calls:
            idx16[:, off // 16 : (off + sz) // 16] = (
                idx_slots[off : off + sz].reshape(sz // 16, 16).T
            )
        idx16 = np.tile(idx16, (NCORES, 1))
        # host-built S_T stream: [128 edge-partitions, nchunk*128 dest cols] fp16
        st = np.zeros((nchunk, P, P), np.float16)
        st[slot // P, slot % P, rowm[m][perm]] = vals[m][perm]
        st = np.ascontiguousarray(st.transpose(1, 0, 2).reshape(P, nchunk * P))
        per_core.append({"idx16": idx16, "st": st})

    struct = {
        "shard": shard,
        "shard_pad": shard_pad,
        "npad": npad,
        "qrows": qrows,
        "T": T,
        "banks": banks,
        "C": C,
        "calls": calls,
        "S": S,
        "nchunk": nchunk,
        "core_of": core_of,
        "pos_of": pos_of,
    }
    return struct, per_core


def _build_nc(st):
    import concourse.mybir as mybir
    import concourse.tile as tile
    from concourse import bacc
    from concourse.masks import make_identity

    f32 = mybir.dt.float32
    f16 = mybir.dt.float16
    i16 = mybir.dt.int16
    AF = mybir.ActivationFunctionType

    shard_pad, npad, qrows = st["shard_pad"], st["npad"], st["qrows"]
    banks, C, calls = st["banks"], st["C"], st["calls"]
    S, nchunk = st["S"], st["nchunk"]

    nc = bacc.Bacc(None, target_bir_lowering=False, num_swdge_queues=4)

    x_fm = nc.dram_tensor("x_fm", [P, shard_pad], f16, kind="ExternalInput")
    w_in = nc.dram_tensor("w_in", [P, P], f16, kind="ExternalInput")
    b_in = nc.dram_tensor("b_in", [P, 1], f32, kind="ExternalInput")
    w1 = nc.dram_tensor("w1", [P, P], f16, kind="ExternalInput")
    w2 = nc.dram_tensor("w2", [P, P], f16, kind="ExternalInput")
    w_out = nc.dram_tensor("w_out", [P, 1], f16, kind="ExternalInput")
    idx16_d = nc.dram_tensor("idx16", [P, S // 16], i16, kind="ExternalInput")
    st_d = nc.dram_tensor("st", [P, nchunk * P], f16, kind="ExternalInput")
    y_d = nc.dram_tensor("y", [1, shard_pad], f32, kind="ExternalOutput")
    cc_h_in = nc.dram_tensor("cc_h_in", [shard_pad, P], f16)
    h_tab = nc.dram_tensor("h_tab", [npad, P], f16, addr_space="Shared")
    cc_ah_in = nc.dram_tensor("cc_ah_in", [shard_pad, P], f16)
    ah_tab = nc.dram_tensor("ah_tab", [npad, P], f16, addr_space="Shared")
    rg = [list(range(NCORES))]

    gmax = max(sz for _, _, sz in calls)

    with tile.TileContext(nc) as tc:
        with (
            tc.tile_pool(name="const", bufs=1) as cp,
            tc.tile_pool(name="meta", bufs=1) as mp,
            tc.tile_pool(name="fm", bufs=1) as fmp,
            tc.tile_pool(name="xw", bufs=3) as xp,
            tc.tile_pool(name="hw", bufs=3) as hp,
            tc.tile_pool(name="nm", bufs=4) as nmp,
            tc.tile_pool(name="stb", bufs=5) as stp,
            tc.tile_pool(name="g", bufs=10) as gp,
            tc.tile_pool(name="ps_mm", bufs=4, space="PSUM") as pmm,
            tc.tile_pool(name="ps_tp", bufs=2, space="PSUM") as ptp,
            tc.tile_pool(name="ps_o", bufs=2, space="PSUM") as pso,
        ):
            t_ident = cp.tile([P, P], f16, tag="ident")
            make_identity(nc, t_ident[:])
            t_w_in = cp.tile([P, P], f16, tag="w_in")
            nc.sync.dma_start(out=t_w_in[:], in_=w_in[:])
            t_b_in = cp.tile([P, 1], f32, tag="b_in")
            nc.sync.dma_start(out=t_b_in[:], in_=b_in[:])
            t_w1 = cp.tile([P, P], f16, tag="w1")
            nc.sync.dma_start(out=t_w1[:], in_=w1[:])
            t_w2 = cp.tile([P, P], f16, tag="w2")
            nc.sync.dma_start(out=t_w2[:], in_=w2[:])
            t_wout = cp.tile([P, 1], f16, tag="wout")
            nc.sync.dma_start(out=t_wout[:], in_=w_out[:])
            t_idx = mp.tile([P, S // 16], i16, tag="idx")
            nc.sync.dma_start(out=t_idx[:], in_=idx16_d[:])
            ah_fm = fmp.tile([P, shard_pad], f16, tag="ah_fm")
            a2h_fm = fmp.tile([P, shard_pad], f16, tag="a2h_fm")

            def piece_cc(cc_in, tab, p):
                # piece-major table: piece p holds rows [p*8*QB, (p+1)*8*QB) =
                # concat over cores of local rows [p*QB, (p+1)*QB), so the
                # AllGather output is a contiguous block. Also aligns pieces
                # with gather quartiles: hop-2's quartile-q calls only depend
                # on piece q of the table.
                nc.gpsimd.collective_compute(
                    "AllGather",
                    mybir.AluOpType.bypass,
                    replica_groups=rg,
                    ins=[cc_in[p * QB : (p + 1) * QB, :]],
                    outs=[tab[p * NCORES * QB : (p + 1) * NCORES * QB, :]],
                )

            # ---- H = relu(X @ W_in + b_in), feature-major; emit node-major table
            # H-table pieces fire DURING the H phase (overlaps compute, which is
            # PE/Scalar-bound, so no DMA contention) -> hop 1 starts early
            next_piece = [0]
            for j0 in range(0, shard_pad, FJ):
                w = min(FJ, shard_pad - j0)
                xt = xp.tile([P, FJ], f16, tag="x")
                nc.sync.dma_start(out=xt[:, :w], in_=x_fm[:, j0 : j0 + w])
                ps = pmm.tile([P, FJ], f32, tag="mm")
                nc.tensor.matmul(
                    out=ps[:, :w], lhsT=t_w_in[:], rhs=xt[:, :w], start=True, stop=True
                )
                ht = hp.tile([P, FJ], f16, tag="h")
                nc.scalar.activation(
                    ht[:, :w], ps[:, :w], AF.Relu, bias=t_b_in[:, :1], scale=1.0
                )
                for i0 in range(0, w, P):
                    pst = ptp.tile([P, P], f16, tag="tp")
                    nc.tensor.transpose(
                        out=pst[:], in_=ht[:, i0 : i0 + P], identity=t_ident[:]
                    )
                    nmt = nmp.tile([P, P], f16, tag="nm")
                    nc.scalar.copy(nmt[:], pst[:])
                    nc.sync.dma_start(
                        out=cc_h_in[j0 + i0 : j0 + i0 + P, :], in_=nmt[:]
                    )
                while next_piece[0] < 4 and j0 + w >= (next_piece[0] + 1) * QB:
                    piece_cc(cc_h_in, h_tab, next_piece[0])
                    next_piece[0] += 1

            # ---- SpMM pass over the edge stream
            state = {"chunk": 0, "call": 0, "ncall": 0}

            def spmm(src_tab, out_fm, nm_out, cc_tab=None, dense=None):
                piece_start = 0
                for tiles in banks:
                    ps = pmm.tile([P, FJ], f32, tag="mm")
                    total = int(sum(C[tt * 4 + qq] for tt in tiles for qq in range(4)))
                    done = 0
                    for qq in range(4):
                        nch = int(sum(C[tt * 4 + qq] for tt in tiles))
                        sz = nch * P
                        if sz == 0:
                            continue
                        cq, off, csz = calls[state["call"]]
                        assert cq == qq and csz == sz
                        state["call"] += 1
                        g = gp.tile([P, gmax], f16, tag="g")
                        nc.gpsimd.dma_gather(
                            out_ap=g[:, :sz].rearrange("p (c d) -> p c d", d=P),
                            in_ap=src_tab[qq * qrows : (qq + 1) * qrows, :],
                            idxs_ap=t_idx[:, off // 16 : (off + sz) // 16],
                            num_idxs=sz,
                            num_idxs_reg=sz,
                            elem_size=P,
                            single_packet=False,
                            queue_num=state["ncall"] % 4,
                        )
                        state["ncall"] += 1
                        stt = stp.tile([P, gmax], f16, tag="stb")
                        c0 = state["chunk"]
                        nc.sync.dma_start(
                            out=stt[:, :sz], in_=st_d[:, c0 * P : c0 * P + sz]
                        )
                        pos = 0
                        for tt in tiles:
                            for _c in range(int(C[tt * 4 + qq])):
                                ti = tt - tiles[0]
                                nc.tensor.matmul(
                                    out=ps[:, ti * P : (ti + 1) * P],
                                    lhsT=g[:, pos * P : (pos + 1) * P],
                                    rhs=stt[:, pos * P : (pos + 1) * P],
                                    start=(done == 0),
                                    stop=(done == total - 1),
                                )
                                done += 1
                                pos += 1
                                state["chunk"] += 1
                    w = len(tiles) * P
                    f0 = tiles[0] * P
                    nc.scalar.copy(out_fm[:, f0 : f0 + w], ps[:, :w])
                    if nm_out is not None:
                        for tt in tiles:
                            pst = ptp.tile([P, P], f16, tag="tp")
                            nc.tensor.transpose(
                                out=pst[:],
                                in_=out_fm[:, tt * P : (tt + 1) * P],
                                identity=t_ident[:],
                            )
                            nmt = nmp.tile([P, P], f16, tag="nm")
                            nc.scalar.copy(nmt[:], pst[:])
                            nc.sync.dma_start(
                                out=nm_out[tt * P : (tt + 1) * P, :], in_=nmt[:]
                            )
                    if dense is not None:
                        dense(f0, w)

            # ---- H2 = relu(W1.T@AH + W2.T@A2H); y = W_out.T @ H2 (per bank)
            def dense_tail(j0, w):
                ps = pmm.tile([P, FJ], f32, tag="mm")
                nc.tensor.matmul(
                    out=ps[:, :w],
                    lhsT=t_w1[:],
                    rhs=ah_fm[:, j0 : j0 + w],
                    start=True,
                    stop=False,
                )
                nc.tensor.matmul(
                    out=ps[:, :w],
                    lhsT=t_w2[:],
                    rhs=a2h_fm[:, j0 : j0 + w],
                    start=False,
                    stop=True,
                )
                h2 = hp.tile([P, FJ], f16, tag="h2")
                nc.scalar.activation(h2[:, :w], ps[:, :w], AF.Relu)
                ps2 = pso.tile([P, FJ], f32, tag="o")
                nc.tensor.matmul(
                    out=ps2[:1, :w], lhsT=t_wout[:, :1], rhs=h2[:, :w], start=True, stop=True
                )
                yt = nmp.tile([1, FJ], f32, tag="y")
                nc.scalar.copy(yt[:1, :w], ps2[:1, :w])
                nc.sync.dma_start(out=y_d[0:1, j0 : j0 + w], in_=yt[:1, :w])

            spmm(h_tab, ah_fm, cc_ah_in)
            # AH pieces fire back-to-back in the quiet window after hop 1
            # (in-hop collectives steal drain bandwidth 1:1 -- measured); the
            # piece<->quartile alignment still lets hop 2's quartile-q calls
            # start as soon as piece q lands.
            for p in range(4):
                piece_cc(cc_ah_in, ah_tab, p)
            state["chunk"] = 0
            state["call"] = 0
            spmm(ah_tab, a2h_fm, None, dense=dense_tail)

    nc.finalize()
    return nc


def _make_in_maps(inputs, st, per_core):
    shard_pad = st["shard_pad"]
    core_of, pos_of = st["core_of"], st["pos_of"]
    X = np.asarray(inputs["X"], np.float32)
    W_in = np.ascontiguousarray(np.asarray(inputs["W_in"], np.float32).astype(np.float16))
    b_in = np.asarray(inputs["b_in"], np.float32).reshape(P, 1)
    w1 = np.asarray(inputs["W_mp1"], np.float32).astype(np.float16)
    w2 = np.asarray(inputs["W_mp2"], np.float32).astype(np.float16)
    w_out = np.asarray(inputs["W_out"], np.float32).astype(np.float16).reshape(P, 1)
    in_maps = []
    for k in range(NCORES):
        m = core_of == k
        x_fm = np.zeros((P, shard_pad), np.float16)
        x_fm[:, pos_of[m]] = X[m].T.astype(np.float16)
        in_maps.append(
            {
                "x_fm": x_fm,
                "w_in": W_in,
                "b_in": b_in,
                "w1": np.ascontiguousarray(w1),
                "w2": np.ascontiguousarray(w2),
                "w_out": np.ascontiguousarray(w_out),
                "idx16": per_core[k]["idx16"],
                "st": per_core[k]["st"],
            }
        )
    return in_maps


def kernel(**inputs):
    from concourse.bass_utils import run_bass_kernel_spmd

    row = np.asarray(inputs["row"], np.int64)
    col = np.asarray(inputs["col"], np.int64)
    vals = np.asarray(inputs["vals"], np.float32)
    n_nodes = int(np.asarray(inputs["X"]).shape[0])

    st, per_core = _structure(row, col, vals, n_nodes)
    nc = _build_nc(st)
    in_maps = _make_in_maps(inputs, st, per_core)

    trace = bool(int(os.environ.get("GNN_TRACE", "0")))
    res = run_bass_kernel_spmd(
        nc, in_maps, core_ids=list(range(NCORES)), trace=trace
    )
    if trace:
        kernel.last_exec_time_ns = res.exec_time_ns

    b_out = float(np.asarray(inputs["b_out"]).reshape(-1)[0])
    core_of, pos_of = st["core_of"], st["pos_of"]
    ys = np.stack([res.results[k]["y"][0] for k in range(NCORES)])
    out = ys[core_of, pos_of].astype(np.float32)
    return (out + b_out).reshape(n_nodes, 1)



# revision 42
# speedup vs baseline: 1.5848x; 1.5848x over previous
"""BetaGNN message-passing kernel for 8 Trainium2 NeuronCores.

Strategy (1D node partitioning):
  - nodes are PERMUTED across cores/tiles by a host-side balancer (_balance):
    LPT on dest degree equalizes per-core edge counts exactly, then a 4-dim
    LPT + swap-repair keeps every per-(core,tile,quartile) cell count under a
    shared 512 cap (spill tiles get 640), so nearly every cell needs exactly
    ceil(512/128)=4 gather chunks: 201,344 slots/core/hop vs 249,856 naive
    (-19.4% gather packets and one-hot stream bytes)
  - H = relu(X @ W_in + b_in) computed per-shard feature-major (X and W_in in
    fp16), transposed on PE into a node-major fp16 gather table, AllGathered
  - SpMM (y[r] = sum vals[e]*H[col[e]] over edges with row[e]==r): edges are
    chunked 128-at-a-time per (dest tile, source quartile) cell, slots sorted
    by source id within each cell (ascending HBM addresses raised the SDMA
    drain rate from 3.7 to 2.4 ns/packet); source rows are fetched with the
    GPSIMD dma_gather custom op (int16 indices over 4 quarter-tables, 4 SWDGE
    queues). The kernel is bound by the SDMA drain of these random 256B reads
    (~107 GB/s aggregate), not by descriptor generation. The scaled one-hot
    selection matrices S_T[e,d] = vals[e]*(rowm[e]==d) are HOST-precomputed
    fp16 and streamed via HWDGE; TensorE computes G.T @ S_T accumulating AH
    feature-major in PSUM (one bank per 4 dest tiles)
  - AH is AllGathered (fp16 table) for the second hop in a quiet window
    (overlapping collectives with gather drains steals bandwidth 1:1); the
    same S_T stream is re-read for hop 2; A2H stays feature-major
  - H2 = relu(W1.T@AH + W2.T@A2H) and y = W_out.T@H2 are computed per bank,
    interleaved into hop 2 as each bank's A2H lands; b_out added on host

The edge chunk structure (chunks per cell) is data-dependent and baked into
the compiled program; all 8 cores share one program, so per-cell chunk counts
are the max over cores and each core pads its edge stream with (idx=0, val=0)
slots (~0.7% after balancing).
"""

import os
import sys

import numpy as np

if "/opt/trn_rl_repo" not in sys.path:
    sys.path.insert(0, "/opt/trn_rl_repo")

NCORES = 8
P = 128
FJ = 512  # dense-phase column chunk
BANK_TILES = 4  # dest tiles per PSUM bank


def _balance(row, col, n_nodes, shard, T):
    """Permute nodes across cores/tiles so per-(core,tile,quartile) edge
    counts stay under a shared cap -> chunk counts near the ceil-optimal.

    Returns (core_of, pos_of): node -> owning core, local slot in [0, T*128).
    """
    deg_in = np.bincount(row, minlength=n_nodes)

    # nodes -> cores: LPT on dest degree with exact shard capacity
    order = np.argsort(-deg_in, kind="stable")
    core_of = np.empty(n_nodes, np.int32)
    core_load = np.zeros(NCORES, np.int64)
    core_cnt = np.zeros(NCORES, np.int64)
    for n in order:
        k = np.argmin(np.where(core_cnt < shard, core_load, 1 << 60))
        core_of[n] = k
        core_load[k] += deg_in[n]
        core_cnt[k] += 1

    quart_of = core_of // 2
    qd = np.zeros((n_nodes, 4), np.int64)
    np.add.at(qd, (row, quart_of[col]), 1)

    tot_kq = np.zeros((NCORES, 4), np.int64)
    for k in range(NCORES):
        m = core_of[row] == k
        tot_kq[k] = np.bincount(quart_of[col[m]], minlength=4)
    over = np.maximum(0, tot_kq - T * 512)
    n_spill = -(-over.max(axis=0) // P)
    caps = np.full((T, 4), 512, np.int64)
    for q in range(4):
        caps[: n_spill[q], q] = 640

    pos_of = np.empty(n_nodes, np.int32)
    for k in range(NCORES):
        nodes = np.where(core_of == k)[0]
        d = qd[nodes]
        target = d.sum(axis=0) / T
        o = np.argsort(-d.sum(axis=1), kind="stable")
        nodes, d = nodes[o], d[o]
        tile_sum = np.zeros((T, 4), np.int64)
        tile_cnt = np.zeros(T, np.int64)
        assign = np.empty(len(nodes), np.int32)
        for i in range(len(nodes)):
            proj = tile_sum + d[i]
            score = (proj - caps).max(axis=1).astype(np.float64)
            score += 0.001 * (proj - target).max(axis=1)
            score = np.where(tile_cnt < P, score, 1e18)
            t = int(np.argmin(score))
            assign[i] = t
            tile_sum[t] += d[i]
            tile_cnt[t] += 1
        members = [list(np.where(assign == t)[0]) for t in range(T)]
        for _ in range(20000):
            viol = tile_sum - caps
            vq = np.unravel_index(np.argmax(viol), viol.shape)
            if viol[vq] <= 0:
                break
            t, q = int(vq[0]), int(vq[1])
            done = False
            rows_t = sorted(members[t], key=lambda i: -d[i][q])
            for a in rows_t[:20]:
                if d[a][q] == 0:
                    break
                slack = caps[:, q] - tile_sum[:, q]
                for u in np.argsort(-slack):
                    if slack[u] <= 0:
                        break
                    if u == t:
                        continue
                    for b in members[u]:
                        if d[b][q] >= d[a][q]:
                            continue
                        nt = tile_sum[t] - d[a] + d[b]
                        nu = tile_sum[u] - d[b] + d[a]
                        if (nt <= caps[t]).all() or (nt - caps[t]).max() < viol[vq]:
                            if (nu <= caps[u]).all():
                                tile_sum[t] = nt
                                tile_sum[u] = nu
                                members[t].remove(a)
                                members[u].remove(b)
                                members[t].append(b)
                                members[u].append(a)
                                assign[a], assign[b] = u, t
                                done = True
                                break
                    if done:
                        break
                if done:
                    break
            if not done:
                break
        for t in range(T):
            for j, i in enumerate(members[t]):
                pos_of[nodes[i]] = t * P + j
    return core_of, pos_of


def _structure(row, col, vals, n_nodes):
    """Shared (cross-core) chunk structure + per-core padded edge arrays."""
    shard = n_nodes // NCORES
    shard_pad = -(-shard // P) * P
    npad = shard_pad * NCORES
    qrows = npad // 4
    assert qrows <= 32768, qrows
    T = shard_pad // P
    banks = [list(range(i, min(i + BANK_TILES, T))) for i in range(0, T, BANK_TILES)]

    core_of, pos_of = _balance(row, col, n_nodes, shard, T)

    core = core_of[row]
    r_loc = pos_of[row]
    gcol = core_of[col] * shard_pad + pos_of[col]
    q = gcol // qrows
    idxq = (gcol - q * qrows).astype(np.int16)
    t = r_loc // P
    rowm = (r_loc % P).astype(np.int32)

    ncells = T * 4
    cell = t * 4 + q
    cnt = np.zeros((NCORES, ncells), np.int64)
    for k in range(NCORES):
        m = core == k
        cnt[k] = np.bincount(cell[m], minlength=ncells)
    C = -(-cnt.max(axis=0) // P)  # chunks per cell (ceil of max count / 128)
    # every tile must own >= 1 chunk so its PSUM window gets written
    for tt in range(T):
        if C[tt * 4 : tt * 4 + 4].sum() == 0:
            C[tt * 4] = 1

    # stream order: for bank, for quartile, for tile-in-bank
    order_cells = [tt * 4 + qq for b in banks for qq in range(4) for tt in b]
    ord_of_cell = np.empty(ncells, np.int64)
    ord_of_cell[np.array(order_cells)] = np.arange(ncells)
    cell_chunks = C[np.array(order_cells)]
    cell_slot0 = np.r_[0, np.cumsum(cell_chunks * P)[:-1]]
    S = int((cell_chunks * P).sum())
    nchunk = S // P

    # gather calls: one per (bank, quartile) with nonzero size
    calls = []  # (quartile, slot_off, size)
    pos = 0
    for b in banks:
        for qq in range(4):
            sz = int(sum(C[tt * 4 + qq] for tt in b)) * P
            if sz:
                calls.append((qq, pos, sz))
                pos += sz
    assert pos == S

    per_core = []
    for k in range(NCORES):
        m = core == k
        ek = ord_of_cell[cell[m]]
        # order slots by (cell stream order, then ascending source idx) --
        # ascending sources improve HBM row locality during the gather
        perm = np.lexsort((idxq[m], ek))
        sorted_ord = ek[perm]
        counts_in_order = cnt[k][np.array(order_cells)]
        run_start = np.r_[0, np.cumsum(counts_in_order)[:-1]]
        rank = np.arange(len(sorted_ord)) - run_start[sorted_ord]
        slot = cell_slot0[sorted_ord] + rank
        idx_slots = np.zeros(S, np.int16)
        idx_slots[slot] = idxq[m][perm]
        # int16 index array: per call, wrap [sz] -> [16, sz/16]; replicate to 128
        idx16 = np.zeros((16, S // 16), np.int16)
        for qq, off, sz in calls:
            idx16[:, off // 16 : (off + sz) // 16] = (
                idx_slots[off : off + sz].reshape(sz // 16, 16).T
            )
        idx16 = np.tile(idx16, (NCORES, 1))
        # host-built S_T stream: [128 edge-partitions, nchunk*128 dest cols] fp16
        st = np.zeros((nchunk, P, P), np.float16)
        st[slot // P, slot % P, rowm[m][perm]] = vals[m][perm]
        st = np.ascontiguousarray(st.transpose(1, 0, 2).reshape(P, nchunk * P))
        per_core.append({"idx16": idx16, "st": st})

    struct = {
        "shard": shard,
        "shard_pad": shard_pad,
        "npad": npad,
        "qrows": qrows,
        "T": T,
        "banks": banks,
        "C": C,
        "calls": calls,
        "S": S,
        "nchunk": nchunk,
        "core_of": core_of,
        "pos_of": pos_of,
    }
    return struct, per_core


def _build_nc(st):
    import concourse.mybir as mybir
    import concourse.tile as tile
    from concourse import bacc
    from concourse.masks import make_identity

    f32 = mybir.dt.float32
    f16 = mybir.dt.float16
    i16 = mybir.dt.int16
    AF = mybir.ActivationFunctionType

    shard_pad, npad, qrows = st["shard_pad"], st["npad"], st["qrows"]
    banks, C, calls = st["banks"], st["C"], st["calls"]
    S, nchunk = st["S"], st["nchunk"]

    nc = bacc.Bacc(None, target_bir_lowering=False, num_swdge_queues=4)

    x_fm = nc.dram_tensor("x_fm", [P, shard_pad], f16, kind="ExternalInput")
    w_in = nc.dram_tensor("w_in", [P, P], f16, kind="ExternalInput")
    b_in = nc.dram_tensor("b_in", [P, 1], f32, kind="ExternalInput")
    w1 = nc.dram_tensor("w1", [P, P], f16, kind="ExternalInput")
    w2 = nc.dram_tensor("w2", [P, P], f16, kind="ExternalInput")
    w_out = nc.dram_tensor("w_out", [P, 1], f16, kind="ExternalInput")
    idx16_d = nc.dram_tensor("idx16", [P, S // 16], i16, kind="ExternalInput")
    st_d = nc.dram_tensor("st", [P, nchunk * P], f16, kind="ExternalInput")
    y_d = nc.dram_tensor("y", [1, shard_pad], f32, kind="ExternalOutput")
    cc_h_in = nc.dram_tensor("cc_h_in", [shard_pad, P], f16)
    h_tab = nc.dram_tensor("h_tab", [npad, P], f16, addr_space="Shared")
    cc_ah_in = nc.dram_tensor("cc_ah_in", [shard_pad, P], f16)
    ah_tab = nc.dram_tensor("ah_tab", [npad, P], f16, addr_space="Shared")
    rg = [list(range(NCORES))]

    gmax = max(sz for _, _, sz in calls)

    with tile.TileContext(nc) as tc:
        with (
            tc.tile_pool(name="const", bufs=1) as cp,
            tc.tile_pool(name="meta", bufs=1) as mp,
            tc.tile_pool(name="fm", bufs=1) as fmp,
            tc.tile_pool(name="xw", bufs=3) as xp,
            tc.tile_pool(name="hw", bufs=3) as hp,
            tc.tile_pool(name="nm", bufs=4) as nmp,
            tc.tile_pool(name="stb", bufs=5) as stp,
            tc.tile_pool(name="g", bufs=10) as gp,
            tc.tile_pool(name="ps_mm", bufs=4, space="PSUM") as pmm,
            tc.tile_pool(name="ps_tp", bufs=2, space="PSUM") as ptp,
            tc.tile_pool(name="ps_o", bufs=2, space="PSUM") as pso,
        ):
            t_ident = cp.tile([P, P], f16, tag="ident")
            make_identity(nc, t_ident[:])
            t_w_in = cp.tile([P, P], f16, tag="w_in")
            nc.sync.dma_start(out=t_w_in[:], in_=w_in[:])
            t_b_in = cp.tile([P, 1], f32, tag="b_in")
            nc.sync.dma_start(out=t_b_in[:], in_=b_in[:])
            t_w1 = cp.tile([P, P], f16, tag="w1")
            nc.sync.dma_start(out=t_w1[:], in_=w1[:])
            t_w2 = cp.tile([P, P], f16, tag="w2")
            nc.sync.dma_start(out=t_w2[:], in_=w2[:])
            t_wout = cp.tile([P, 1], f16, tag="wout")
            nc.sync.dma_start(out=t_wout[:], in_=w_out[:])
            t_idx = mp.tile([P, S // 16], i16, tag="idx")
            nc.sync.dma_start(out=t_idx[:], in_=idx16_d[:])
            ah_fm = fmp.tile([P, shard_pad], f16, tag="ah_fm")
            a2h_fm = fmp.tile([P, shard_pad], f16, tag="a2h_fm")

            def full_cc(cc_in, tab):
                nc.gpsimd.collective_compute(
                    "AllGather",
                    mybir.AluOpType.bypass,
                    replica_groups=rg,
                    ins=[cc_in[:]],
                    outs=[tab[:]],
                )

            # ---- H = relu(X @ W_in + b_in), feature-major; emit node-major table
            for j0 in range(0, shard_pad, FJ):
                w = min(FJ, shard_pad - j0)
                xt = xp.tile([P, FJ], f16, tag="x")
                nc.sync.dma_start(out=xt[:, :w], in_=x_fm[:, j0 : j0 + w])
                ps = pmm.tile([P, FJ], f32, tag="mm")
                nc.tensor.matmul(
                    out=ps[:, :w], lhsT=t_w_in[:], rhs=xt[:, :w], start=True, stop=True
                )
                ht = hp.tile([P, FJ], f16, tag="h")
                nc.scalar.activation(
                    ht[:, :w], ps[:, :w], AF.Relu, bias=t_b_in[:, :1], scale=1.0
                )
                for i0 in range(0, w, P):
                    pst = ptp.tile([P, P], f16, tag="tp")
                    nc.tensor.transpose(
                        out=pst[:], in_=ht[:, i0 : i0 + P], identity=t_ident[:]
                    )
                    nmt = nmp.tile([P, P], f16, tag="nm")
                    nc.scalar.copy(nmt[:], pst[:])
                    nc.sync.dma_start(
                        out=cc_h_in[j0 + i0 : j0 + i0 + P, :], in_=nmt[:]
                    )
            full_cc(cc_h_in, h_tab)

            # ---- SpMM pass over the edge stream
            state = {"chunk": 0, "call": 0, "ncall": 0}

            def spmm(src_tab, out_fm, nm_out, cc_tab=None, dense=None):
                piece_start = 0
                for tiles in banks:
                    ps = pmm.tile([P, FJ], f32, tag="mm")
                    total = int(sum(C[tt * 4 + qq] for tt in tiles for qq in range(4)))
                    done = 0
                    for qq in range(4):
                        nch = int(sum(C[tt * 4 + qq] for tt in tiles))
                        sz = nch * P
                        if sz == 0:
                            continue
                        cq, off, csz = calls[state["call"]]
                        assert cq == qq and csz == sz
                        state["call"] += 1
                        g = gp.tile([P, gmax], f16, tag="g")
                        nc.gpsimd.dma_gather(
                            out_ap=g[:, :sz].rearrange("p (c d) -> p c d", d=P),
                            in_ap=src_tab[qq * qrows : (qq + 1) * qrows, :],
                            idxs_ap=t_idx[:, off // 16 : (off + sz) // 16],
                            num_idxs=sz,
                            num_idxs_reg=sz,
                            elem_size=P,
                            single_packet=False,
                            queue_num=state["ncall"] % 4,
                        )
                        state["ncall"] += 1
                        stt = stp.tile([P, gmax], f16, tag="stb")
                        c0 = state["chunk"]
                        nc.sync.dma_start(
                            out=stt[:, :sz], in_=st_d[:, c0 * P : c0 * P + sz]
                        )
                        pos = 0
                        for tt in tiles:
                            for _c in range(int(C[tt * 4 + qq])):
                                ti = tt - tiles[0]
                                nc.tensor.matmul(
                                    out=ps[:, ti * P : (ti + 1) * P],
                                    lhsT=g[:, pos * P : (pos + 1) * P],
                                    rhs=stt[:, pos * P : (pos + 1) * P],
                                    start=(done == 0),
                                    stop=(done == total - 1),
                                )
                                done += 1
                                pos += 1
                                state["chunk"] += 1
                    w = len(tiles) * P
                    f0 = tiles[0] * P
                    nc.scalar.copy(out_fm[:, f0 : f0 + w], ps[:, :w])
                    if nm_out is not None:
                        for tt in tiles:
                            pst = ptp.tile([P, P], f16, tag="tp")
                            nc.tensor.transpose(
                                out=pst[:],
                                in_=out_fm[:, tt * P : (tt + 1) * P],
                                identity=t_ident[:],
                            )
                            nmt = nmp.tile([P, P], f16, tag="nm")
                            nc.scalar.copy(nmt[:], pst[:])
                            nc.sync.dma_start(
                                out=nm_out[tt * P : (tt + 1) * P, :], in_=nmt[:]
                            )
                    if dense is not None:
                        dense(f0, w)

            # ---- H2 = relu(W1.T@AH + W2.T@A2H); y = W_out.T @ H2 (per bank)
            def dense_tail(j0, w):
                ps = pmm.tile([P, FJ], f32, tag="mm")
                nc.tensor.matmul(
                    out=ps[:, :w],
                    lhsT=t_w1[:],
                    rhs=ah_fm[:, j0 : j0 + w],
                    start=True,
                    stop=False,
                )
                nc.tensor.matmul(
                    out=ps[:, :w],
                    lhsT=t_w2[:],
                    rhs=a2h_fm[:, j0 : j0 + w],
                    start=False,
                    stop=True,
                )
                h2 = hp.tile([P, FJ], f16, tag="h2")
                nc.scalar.activation(h2[:, :w], ps[:, :w], AF.Relu)
                ps2 = pso.tile([P, FJ], f32, tag="o")
                nc.tensor.matmul(
                    out=ps2[:1, :w], lhsT=t_wout[:, :1], rhs=h2[:, :w], start=True, stop=True
                )
                yt = nmp.tile([1, FJ], f32, tag="y")
                nc.scalar.copy(yt[:1, :w], ps2[:1, :w])
                nc.sync.dma_start(out=y_d[0:1, j0 : j0 + w], in_=yt[:1, :w])

            spmm(h_tab, ah_fm, cc_ah_in)
            full_cc(cc_ah_in, ah_tab)
            state["chunk"] = 0
            state["call"] = 0
            spmm(ah_tab, a2h_fm, None, dense=dense_tail)

    nc.finalize()
    return nc


def _make_in_maps(inputs, st, per_core):
    shard_pad = st["shard_pad"]
    core_of, pos_of = st["core_of"], st["pos_of"]
    X = np.asarray(inputs["X"], np.float32)
    W_in = np.ascontiguousarray(np.asarray(inputs["W_in"], np.float32).astype(np.float16))
    b_in = np.asarray(inputs["b_in"], np.float32).reshape(P, 1)
    w1 = np.asarray(inputs["W_mp1"], np.float32).astype(np.float16)
    w2 = np.asarray(inputs["W_mp2"], np.float32).astype(np.float16)
    w_out = np.asarray(inputs["W_out"], np.float32).astype(np.float16).reshape(P, 1)
    in_maps = []
    for k in range(NCORES):
        m = core_of == k
        x_fm = np.zeros((P, shard_pad), np.float16)
        x_fm[:, pos_of[m]] = X[m].T.astype(np.float16)
        in_maps.append(
            {
                "x_fm": x_fm,
                "w_in": W_in,
                "b_in": b_in,
                "w1": np.ascontiguousarray(w1),
                "w2": np.ascontiguousarray(w2),
                "w_out": np.ascontiguousarray(w_out),
                "idx16": per_core[k]["idx16"],
                "st": per_core[k]["st"],
            }
        )
    return in_maps


def kernel(**inputs):
    from concourse.bass_utils import run_bass_kernel_spmd

    row = np.asarray(inputs["row"], np.int64)
    col = np.asarray(inputs["col"], np.int64)
    vals = np.asarray(inputs["vals"], np.float32)
    n_nodes = int(np.asarray(inputs["X"]).shape[0])

    st, per_core = _structure(row, col, vals, n_nodes)
    nc = _build_nc(st)
    in_maps = _make_in_maps(inputs, st, per_core)

    trace = bool(int(os.environ.get("GNN_TRACE", "0")))
    res = run_bass_kernel_spmd(
        nc, in_maps, core_ids=list(range(NCORES)), trace=trace
    )
    if trace:
        kernel.last_exec_time_ns = res.exec_time_ns

    b_out = float(np.asarray(inputs["b_out"]).reshape(-1)[0])
    core_of, pos_of = st["core_of"], st["pos_of"]
    ys = np.stack([res.results[k]["y"][0] for k in range(NCORES)])
    out = ys[core_of, pos_of].astype(np.float32)
    return (out + b_out).reshape(n_nodes, 1)

